# revision 88
# baseline (speedup 1.0000x reference)
"""Trainium2 Bass kernel for nn_Attention_37752762532690.

Reference math (B=8, S=2048, H=1024):
    state_trans = broadcast(decoder_state_t) -> (B, S, H)   # rows identical over S
    multip[b,i,j] = <state_trans[b,i,:], enc[b,j,:]>        # independent of i
    raw[b,i] = sum_j multip[b,i,j]                          # constant over i
    attention_scores = softmax(raw, axis=-1) = 1/S exactly  # softmax of a constant
    context[b,:] = sum_s scores[b,s] * enc[b,s,:] = (1/S) * sum_s enc[b,s,:]

Because softmax of a constant vector is exactly uniform (x - max(x) == 0
elementwise, exp(0) == 1, sum == S, and 1/S == 2^-11 is a power of two),
attention_scores == 1/2048 exactly regardless of the dot-product values, and
context is an exact power-of-two scaling of the per-batch sum over S.

Sharding: pure data parallel over the batch dim — core b handles batch b.
Per core: DMA enc[b] (2048x1024 f32, 8 MiB) into SBUF as 15 full tiles of
[128, 1024] plus per-quantum column slices for the final S-rows, with the
input DMAs alternated across both HWDGE rings (sync and scalar) so the two
queues stream concurrently — on hardware the rings share the SDMA pool and
HBM bandwidth (arrivals interleave pairwise at the same aggregate rate),
while the per-DMA queue overheads overlap across rings. The
accumulation chain is column-split across two engines (DVE cols 0:512,
GPSIMD cols 384:1024, separate accumulator tiles) so the work remaining
after the last tiles arrive is split per engine; the final tile needs no
fold stage at all — each 128-col chunk's PSUM column accumulates two
matmuls (accumulator chunk, then last-tile slice chunk). The partition
reduction runs on the PE with the data as the STATIONARY operand and the
1/2048-vector as the 1-column moving operand — cost scales with the moving
side, so all chunk matmuls are nearly free and need no HAM warm-up. The result lands column-major in PSUM ([128 partitions, 8
chunks]), one scalar-engine copy moves it to SBUF, one DMA writes it out,
and the host transposes the 4 KB. Scores are a memset. The kernel is
HBM-bandwidth bound: the 8 MiB/core input stream dominates the ~17.6 us
cost-model time; the tail is the last slice's chunk-matmuls + copy + one
fixed output-DMA latency + the exit barrier.
"""

import numpy as np

import concourse.bacc as bacc
import concourse.mybir as mybir
from concourse.tile import TileContext

B, S, H = 8, 2048, 1024
P = 128
N_TILES = S // P  # 16
INV_S = 1.0 / S  # 2**-11, exact in fp32

_NC_CACHE = None


def _build_nc(
    tail_split=4,
    warm_n=0,
    last_dma_split=1,
    out_split=1,
    warm_src_i=10,
    warm_cols=128,
    tail_widths=(384, 384, 256),
    last_copy_dve=False,
    out_on_act=False,
    gpsimd_folds=(1,),
    split_cols=384,
    tail_order=None,
    tail_tiles=1,
    half_dma_chain_tiles=0,
    batch_plan=None,
    per_quantum_copy=False,
    alt_queues=True,
    memset_eng="gpsimd",
    ring_flip=False,
    slice_ring_alt=True,
    early_slices=0,
    # col_mm=3 (scatter-add output via SWDGE prepare/trigger) is ~1.1us
    # faster in the cost model but produces sporadically wrong results on
    # repeated NEFF executions (SWDGE ring state is not safe across runs).
    col_mm=2,
):
    nc = bacc.Bacc(None)
    f32 = mybir.dt.float32
    scores_memset_eng = getattr(nc, memset_eng)

    enc = nc.dram_tensor("enc", [S, H], f32, kind="ExternalInput")
    if col_mm == 3:
        # Column layout padded to 256-byte rows for dma_scatter_add
        # (stride_bytes must divide by 256); host reads [:, :8].
        # 256 rows: the scatter executor checks every entry of the
        # (partially unread) idx tile against the row count; the iota
        # pattern tops out at 16*7+127=239.
        context_out = nc.dram_tensor(
            "context_out", [256, 64], f32, kind="ExternalOutput"
        )
    elif col_mm:
        # Column layout: context_out[p, c] = context[c*128 + p]; the host
        # transposes (free on 4 KB). Lets the PE reduce with acc as the
        # STATIONARY operand and the 1-column w vector as the moving one.
        context_out = nc.dram_tensor(
            "context_out", [P, H // P], f32, kind="ExternalOutput"
        )
    else:
        context_out = nc.dram_tensor("context_out", [1, H], f32, kind="ExternalOutput")
    scores_out = nc.dram_tensor("scores_out", [1, S], f32, kind="ExternalOutput")

    enc_tiled = enc[:, :].rearrange("(n p) m -> n p m", p=P)

    with TileContext(nc) as tc:
        with (
            tc.tile_pool(name="io", bufs=1) as io_pool,
            tc.tile_pool(name="consts", bufs=1) as const_pool,
            tc.tile_pool(name="psum", bufs=1, space="PSUM") as psum_pool,
        ):
            # Input DMAs first in program order so the sync HWDGE queue
            # starts streaming immediately. The final tile is loaded as
            # separate column-quarter tiles (Tile tracks dependencies per
            # tile, not per region): each tail quantum then unblocks at its
            # own quarter-DMA's stream end instead of waiting for the whole
            # 512 KB tile to land.
            widths = list(tail_widths) if tail_widths else [H // tail_split] * tail_split
            assert sum(widths) == H
            n_chain = N_TILES - tail_tiles
            # Early chain tiles stream as multi-tile batch DMAs: each DMA
            # carries a fixed ~123ns queue gap, so fewer DMAs shorten the
            # stream. Early tiles don't need per-tile arrival granularity —
            # the add chain has slack — only the last few tiles do.
            plan = list(batch_plan) if batch_plan else [1] * n_chain
            assert sum(plan) == n_chain
            tiles = []  # entries: (lo_ap, hi_ap) per chain tile
            deferred_fulls = []
            pos = 0
            for bi, bsz in enumerate(plan):
                if bsz == 1:
                    t = io_pool.tile([P, H], f32, tag=f"in{pos}")
                    odd = (pos % 2 == 0) if ring_flip else (pos % 2 == 1)
                    eng = nc.scalar if (alt_queues and odd) else nc.sync
                    if pos >= n_chain - early_slices:
                        # Defer the last full(s): their DMAs are emitted
                        # after the tail slices so the slices stream first.
                        deferred_fulls.append((eng, t, pos))
                    else:
                        eng.dma_start(out=t[:, :], in_=enc_tiled[pos])
                    tiles.append((t[:, :split_cols], t[:, split_cols:]))
                else:
                    big = io_pool.tile([P, bsz * H], f32, tag=f"bt{bi}")
                    nc.sync.dma_start(
                        out=big[:, :].rearrange("p (n m) -> p n m", m=H),
                        in_=enc[pos * P : (pos + bsz) * P, :].rearrange(
                            "(n p) m -> p n m", p=P
                        ),
                    )
                    for k in range(bsz):
                        tiles.append(
                            (
                                big[:, k * H : k * H + split_cols],
                                big[:, k * H + split_cols : (k + 1) * H],
                            )
                        )
                pos += bsz
            # The tail tiles stream as per-quantum column slices so each
            # quantum's folds unblock as its own slice lands.
            tail_parts = []  # [tile][quantum]
            for ti in range(n_chain, N_TILES):
                parts = []
                j = 0
                for qi, wd in enumerate(widths):
                    lq = io_pool.tile([P, wd], f32, tag=f"t{ti}q{qi}")
                    if slice_ring_alt == 2:
                        sl_eng = nc.scalar if qi % 2 == 0 else nc.sync
                    else:
                        sl_eng = (
                            nc.scalar if (slice_ring_alt and qi % 2) else nc.sync
                        )
                    sl_eng.dma_start(
                        out=lq[:, :], in_=enc_tiled[ti][:, j : j + wd]
                    )
                    parts.append(lq)
                    j += wd
                tail_parts.append(parts)
            for eng, t, pos_ in deferred_fulls:
                eng.dma_start(out=t[:, :], in_=enc_tiled[pos_])

            # attention scores: exactly 1/S everywhere. Memset on DVE (fast
            # start), DMA out on the scalar-engine HWDGE ring so it never
            # blocks the input stream.
            scores_tile = const_pool.tile([1, S], f32)
            scores_memset_eng.memset(scores_tile[:, :], INV_S)
            nc.scalar.dma_start(out=scores_out[:, :], in_=scores_tile[:, :])

            # ones * (1/S) reduction vector (lhsT for the partition matmul).
            w = const_pool.tile([P, 1], f32)
            scores_memset_eng.memset(w[:, :], INV_S)

            # Serial accumulation chain on the vector engine; each add only
            # needs tile i, so the chain advances as DMAs land. The last
            # tile is folded in per H-quarter in the tail below so the PE
            # can start reducing early quarters while later ones finish.
            # The accumulation chain is column-split across two engines:
            # DVE handles cols [0:split_cols], GPSIMD (otherwise idle)
            # handles [split_cols:H] as an independent serial chain. This
            # halves the per-engine work that remains after the last tiles
            # arrive — DVE alone was the end-of-kernel bottleneck.
            sc = split_cols
            # Separate accumulator tiles per engine: Tile tracks deps at
            # tile granularity, so a shared acc would make every tail fold
            # wait on BOTH chains. Split accumulators let the hi folds start
            # as soon as the (faster) gpsimd chain finishes.
            acc_lo = io_pool.tile([P, sc], f32)
            acc_hi = io_pool.tile([P, H - sc], f32)
            nc.vector.tensor_add(
                out=acc_lo[:, :], in0=tiles[0][0], in1=tiles[1][0]
            )
            nc.gpsimd.tensor_add(
                out=acc_hi[:, :], in0=tiles[0][1], in1=tiles[1][1]
            )
            for i in range(2, n_chain):
                nc.vector.tensor_add(
                    out=acc_lo[:, :], in0=acc_lo[:, :], in1=tiles[i][0]
                )
                nc.gpsimd.tensor_add(
                    out=acc_hi[:, :], in0=acc_hi[:, :], in1=tiles[i][1]
                )

            def acc_slice(lo, hi):
                """AP for acc columns [lo:hi) — must lie in one half."""
                if hi <= sc:
                    return acc_lo[:, lo:hi]
                assert lo >= sc
                return acc_hi[:, lo - sc : hi - sc]

            # PE warm-up: keep the tensor engine busy just before the real
            # reduction matmuls so they run at full clock (HAM ramp). Keyed
            # off a late input tile so the scheduler can't run them early.
            warm_psum = psum_pool.tile([1, warm_cols], f32, tag="warm")
            warm_src = tiles[warm_src_i]
            for _ in range(warm_n):
                nc.tensor.matmul(
                    warm_psum[:, :],
                    lhsT=w[:, :],
                    rhs=warm_src[:, 0:warm_cols],
                    start=True,
                    stop=True,
                )

            # Tail, split over H so the final quantum after the last DMA
            # lands is small: add -> PE partition-reduce -> PSUM copy.
            # Separate PSUM tiles per quantum keep each matmul/copy pair in
            # its own bank, so Tile's bank-overlap tracking never serializes
            # a copy against the next matmul.
            if not col_mm:
                psums = []
                for j, wd in enumerate(widths):
                    pq = psum_pool.tile([1, wd], f32, tag=f"ps{j}")
                    psums.append(pq)
                ctx_sbuf = const_pool.tile([1, H], f32)
            if col_mm:
                n_chunks = H // P  # 8 chunks of 128 columns
                if out_split > 1 and col_mm == 1:
                    n_early = n_chunks - widths[-1] // P
                    psum_col = psum_pool.tile([P, n_early], f32, tag="pscol")
                    psum_col_b = psum_pool.tile(
                        [P, n_chunks - n_early], f32, tag="pscolb"
                    )

                    def psum_slot(c):
                        if c < n_early:
                            return psum_col[:, c : c + 1]
                        return psum_col_b[:, c - n_early : c - n_early + 1]
                else:
                    psum_col = psum_pool.tile([P, n_chunks], f32, tag="pscol")

                    def psum_slot(c):
                        return psum_col[:, c : c + 1]
                if col_mm == 3:
                    # Padded column tile + identity token indices for the
                    # SWDGE scatter-add output path (descriptors generated
                    # early; only a cheap trigger sits after the copy).
                    ctx_col = const_pool.tile([P, 64], f32)
                    nc.vector.memset(ctx_col[:, :], 0.0)
                    idxs = const_pool.tile([128, 8], mybir.dt.int16)
                    nc.gpsimd.iota(
                        idxs[:, :], pattern=[[16, 8]], channel_multiplier=1
                    )
                    scatter_sem = nc.alloc_semaphore("ctx_scatter_dma")
                    nc.gpsimd.dma_scatter_add(
                        context_out[:, :],
                        ctx_col[:, :].rearrange("p (r e) -> p r e", r=1),
                        idxs[:, :],
                        P,
                        P,
                        64,
                        prepare_only=True,
                        sem=scatter_sem,
                    )
                else:
                    ctx_col = const_pool.tile([P, n_chunks], f32)
                offsets = list(np.cumsum([0] + widths[:-1]))
                qorder = tail_order if tail_order is not None else range(len(widths))
                for qi in qorder:
                    j, wd = int(offsets[qi]), widths[qi]
                    if col_mm in (1, 3):
                        # Fold, then per-chunk reduce: out[:, c] = acc_chunk.T @ w
                        fold_eng = (
                            nc.gpsimd
                            if (gpsimd_folds is not None and qi in gpsimd_folds)
                            else nc.vector
                        )
                        fold_eng.tensor_add(
                            out=acc_slice(j, j + wd),
                            in0=acc_slice(j, j + wd),
                            in1=tail_parts[0][qi][:, :],
                        )
                        for c in range(j // P, (j + wd) // P):
                            nc.tensor.matmul(
                                psum_slot(c),
                                lhsT=acc_slice(c * P, (c + 1) * P),
                                rhs=w[:, :],
                                start=True,
                                stop=True,
                            )
                        if per_quantum_copy and col_mm == 1 and out_split == 1:
                            c0, c1 = j // P, (j + wd) // P
                            nc.scalar.copy(
                                out=ctx_col[:, c0:c1], in_=psum_col[:, c0:c1]
                            )
                    else:
                        # No DVE fold: accumulate acc-chunk and last-tile
                        # chunk into the same PSUM column.
                        for c in range(j // P, (j + wd) // P):
                            nc.tensor.matmul(
                                psum_col[:, c : c + 1],
                                lhsT=acc_slice(c * P, (c + 1) * P),
                                rhs=w[:, :],
                                start=True,
                                stop=False,
                            )
                            lo = c * P - j
                            nc.tensor.matmul(
                                psum_col[:, c : c + 1],
                                lhsT=tail_parts[0][qi][:, lo : lo + P],
                                rhs=w[:, :],
                                start=False,
                                stop=True,
                            )
                if col_mm == 3:
                    nc.scalar.copy(
                        out=ctx_col[:, :n_chunks], in_=psum_col[:, :]
                    )
                    nc.gpsimd.trigger_dma(count=None)
                elif out_split > 1:
                    # Ship the early chunks while the final quantum's fold
                    # is still pending; separate SBUF tiles keep the first
                    # DMA's dependency off the last copy.
                    ctx_col_b = const_pool.tile([P, n_chunks - n_early], f32)
                    nc.scalar.copy(
                        out=ctx_col[:, :n_early], in_=psum_col[:, :]
                    )
                    nc.sync.dma_start(
                        out=context_out[:, :n_early], in_=ctx_col[:, :n_early]
                    )
                    nc.scalar.copy(out=ctx_col_b[:, :], in_=psum_col_b[:, :])
                    nc.scalar.dma_start(
                        out=context_out[:, n_early:n_chunks], in_=ctx_col_b[:, :]
                    )
                elif per_quantum_copy:
                    nc.sync.dma_start(out=context_out[:, :], in_=ctx_col[:, :])
                else:
                    nc.scalar.copy(out=ctx_col[:, :], in_=psum_col[:, :])
                    nc.sync.dma_start(out=context_out[:, :], in_=ctx_col[:, :])

            offsets = list(np.cumsum([0] + widths[:-1]))
            order = tail_order if tail_order is not None else range(len(widths))
            for qi in order if not col_mm else []:
                j, wd = int(offsets[qi]), widths[qi]
                if gpsimd_folds is not None:
                    on_gpsimd = qi in gpsimd_folds
                else:
                    # Quanta fully inside the GPSIMD column range fold there.
                    on_gpsimd = j >= split_cols
                fold_eng = nc.gpsimd if on_gpsimd else nc.vector
                for parts in tail_parts:
                    fold_eng.tensor_add(
                        out=acc_slice(j, j + wd),
                        in0=acc_slice(j, j + wd),
                        in1=parts[qi][:, :],
                    )
                # Partition-dim reduction: context = (ones/S).T @ acc.
                nc.tensor.matmul(
                    psums[qi][:, :],
                    lhsT=w[:, :],
                    rhs=acc_slice(j, j + wd),
                    start=True,
                    stop=True,
                )
                if last_copy_dve and qi == len(widths) - 1:
                    nc.vector.tensor_copy(
                        out=ctx_sbuf[:, j : j + wd], in_=psums[qi][:, :]
                    )
                else:
                    nc.scalar.copy(out=ctx_sbuf[:, j : j + wd], in_=psums[qi][:, :])
            if not col_mm:
                out_eng = nc.scalar if out_on_act else nc.sync
                if out_split > 1:
                    cut = H - widths[-1]
                    nc.sync.dma_start(
                        out=context_out[:, :cut], in_=ctx_sbuf[:, :cut]
                    )
                    nc.scalar.dma_start(
                        out=context_out[:, cut:], in_=ctx_sbuf[:, cut:]
                    )
                else:
                    out_eng.dma_start(out=context_out[:, :], in_=ctx_sbuf[:, :])

    nc.finalize()
    return nc


def _get_nc():
    global _NC_CACHE
    if _NC_CACHE is None:
        _NC_CACHE = _build_nc()
    return _NC_CACHE


_EXEC_CACHE = None


def _get_exec():
    """Cached jitted SPMD executable (one trace/compile per process).

    Mirrors concourse.bass2jax.run_bass_via_pjrt's multi-core path, but
    reuses the jitted callable across kernel() calls — run_bass_via_pjrt
    builds a fresh closure per call, retracing and recompiling every time.
    """
    global _EXEC_CACHE
    if _EXEC_CACHE is not None:
        return _EXEC_CACHE

    import jax
    from jax.sharding import Mesh, PartitionSpec
    from jax.experimental.shard_map import shard_map

    import concourse.mybir as mybir_
    from concourse import bass2jax

    nc = _get_nc()
    bass2jax.install_neuronx_cc_hook()

    partition_name = nc.partition_id_tensor.name if nc.partition_id_tensor else None
    in_names, out_names, out_avals, zero_out_shapes = [], [], [], []
    for alloc in nc.m.functions[0].allocations:
        if not isinstance(alloc, mybir_.MemoryLocationSet):
            continue
        name = alloc.memorylocations[0].name
        if alloc.kind == "ExternalInput":
            if name != partition_name:
                in_names.append(name)
        elif alloc.kind == "ExternalOutput":
            shape = tuple(alloc.tensor_shape)
            dtype = mybir_.dt.np(alloc.dtype)
            out_names.append(name)
            out_avals.append(jax.core.ShapedArray(shape, dtype))
            zero_out_shapes.append((shape, dtype))
    n_params = len(in_names)
    all_names = list(in_names) + list(out_names)
    if partition_name is not None:
        all_names.append(partition_name)

    def _body(*args):
        operands = list(args)
        if partition_name is not None:
            operands.append(bass2jax.partition_id_tensor())
        outs = bass2jax._bass_exec_p.bind(
            *operands,
            out_avals=tuple(out_avals),
            in_names=tuple(all_names),
            out_names=tuple(out_names),
            lowering_input_output_aliases=(),
            sim_require_finite=True,
            sim_require_nnan=True,
            nc=nc,
        )
        return tuple(outs)

    devices = jax.devices()[:B]
    mesh = Mesh(np.asarray(devices), ("core",))
    n_outs = len(out_names)
    sharded = jax.jit(
        shard_map(
            _body,
            mesh=mesh,
            in_specs=(PartitionSpec("core"),) * (n_params + n_outs),
            out_specs=(PartitionSpec("core"),) * n_outs,
            check_rep=False,
        ),
        donate_argnums=tuple(range(n_params, n_params + n_outs)),
        keep_unused=True,
    )
    _EXEC_CACHE = (sharded, in_names, out_names, zero_out_shapes)
    return _EXEC_CACHE


def kernel(**inputs) -> tuple[np.ndarray, np.ndarray]:
    enc = np.ascontiguousarray(np.asarray(inputs["encoder_outputs"], dtype=np.float32))
    assert enc.shape == (B, S, H)

    sharded, in_names, out_names, zero_out_shapes = _get_exec()
    assert in_names == ["enc"]
    concat_in = [enc.reshape(B * S, H)]
    concat_zeros = [
        np.zeros((B * shape[0], *shape[1:]), dtype) for shape, dtype in zero_out_shapes
    ]
    out_arrs = sharded(*concat_in, *concat_zeros)
    outs = {}
    for i, name in enumerate(out_names):
        shape, _ = zero_out_shapes[i]
        arr = np.asarray(out_arrs[i]).reshape(B, *shape)
        if name == "context_out" and shape != (1, H):
            # Column layout from the swapped-operand PE reduce (possibly
            # padded for the scatter-add path): arr[b, p, c] = context[b,
            # c*128 + p] for p < 128, c < 8.
            arr = np.ascontiguousarray(arr[:, :P, : H // P].transpose(0, 2, 1))
        outs[name] = arr.reshape(B, -1)
    return outs["context_out"], outs["scores_out"]


# revision 90
# speedup vs baseline: 1.0159x; 1.0159x over previous
"""Trainium2 Bass kernel for nn_Attention_37752762532690.

Reference math (B=8, S=2048, H=1024):
    state_trans = broadcast(decoder_state_t) -> (B, S, H)   # rows identical over S
    multip[b,i,j] = <state_trans[b,i,:], enc[b,j,:]>        # independent of i
    raw[b,i] = sum_j multip[b,i,j]                          # constant over i
    attention_scores = softmax(raw, axis=-1) = 1/S exactly  # softmax of a constant
    context[b,:] = sum_s scores[b,s] * enc[b,s,:] = (1/S) * sum_s enc[b,s,:]

Because softmax of a constant vector is exactly uniform (x - max(x) == 0
elementwise, exp(0) == 1, sum == S, and 1/S == 2^-11 is a power of two),
attention_scores == 1/2048 exactly regardless of the dot-product values, and
context is an exact power-of-two scaling of the per-batch sum over S.

Sharding: pure data parallel over the batch dim — core b handles batch b.
Per core: DMA enc[b] (2048x1024 f32, 8 MiB) into SBUF as 15 full tiles of
[128, 1024] plus per-quantum column slices for the final S-rows, with the
input DMAs alternated across both HWDGE rings (sync and scalar) so the two
queues stream concurrently — on hardware the rings share the SDMA pool and
HBM bandwidth (arrivals interleave pairwise at the same aggregate rate),
while the per-DMA queue overheads overlap across rings. The
accumulation chain is column-split across two engines (DVE cols 0:512,
GPSIMD cols 384:1024, separate accumulator tiles) so the work remaining
after the last tiles arrive is split per engine; the final tile needs no
fold stage at all — each 128-col chunk's PSUM column accumulates two
matmuls (accumulator chunk, then last-tile slice chunk). The partition
reduction runs on the PE with the data as the STATIONARY operand and the
1/2048-vector as the 1-column moving operand — cost scales with the moving
side, so all chunk matmuls are nearly free and need no HAM warm-up. The result lands column-major in PSUM ([128 partitions, 8
chunks]), one scalar-engine copy moves it to SBUF, one DMA writes it out,
and the host transposes the 4 KB. Scores are a memset. The kernel is
HBM-bandwidth bound: the 8 MiB/core input stream dominates the ~17.6 us
cost-model time; the tail is the last slice's chunk-matmuls + copy + one
fixed output-DMA latency + the exit barrier.
"""

import numpy as np

import concourse.bacc as bacc
import concourse.mybir as mybir
from concourse.tile import TileContext

B, S, H = 8, 2048, 1024
P = 128
N_TILES = S // P  # 16
INV_S = 1.0 / S  # 2**-11, exact in fp32

_NC_CACHE = None


def _build_nc(
    tail_split=4,
    warm_n=0,
    last_dma_split=1,
    out_split=1,
    warm_src_i=10,
    warm_cols=128,
    tail_widths=(384, 384, 256),
    last_copy_dve=False,
    out_on_act=False,
    gpsimd_folds=(1,),
    split_cols=384,
    tail_order=None,
    tail_tiles=2,
    half_dma_chain_tiles=0,
    batch_plan=None,
    per_quantum_copy=False,
    alt_queues=True,
    memset_eng="gpsimd",
    ring_flip=False,
    slice_ring_alt=True,
    early_slices=0,
    # col_mm=3 (scatter-add output via SWDGE prepare/trigger) is ~1.1us
    # faster in the cost model but produces sporadically wrong results on
    # repeated NEFF executions (SWDGE ring state is not safe across runs).
    col_mm=2,
):
    nc = bacc.Bacc(None)
    f32 = mybir.dt.float32
    scores_memset_eng = getattr(nc, memset_eng)

    enc = nc.dram_tensor("enc", [S, H], f32, kind="ExternalInput")
    if col_mm == 3:
        # Column layout padded to 256-byte rows for dma_scatter_add
        # (stride_bytes must divide by 256); host reads [:, :8].
        # 256 rows: the scatter executor checks every entry of the
        # (partially unread) idx tile against the row count; the iota
        # pattern tops out at 16*7+127=239.
        context_out = nc.dram_tensor(
            "context_out", [256, 64], f32, kind="ExternalOutput"
        )
    elif col_mm:
        # Column layout: context_out[p, c] = context[c*128 + p]; the host
        # transposes (free on 4 KB). Lets the PE reduce with acc as the
        # STATIONARY operand and the 1-column w vector as the moving one.
        context_out = nc.dram_tensor(
            "context_out", [P, H // P], f32, kind="ExternalOutput"
        )
    else:
        context_out = nc.dram_tensor("context_out", [1, H], f32, kind="ExternalOutput")
    scores_out = nc.dram_tensor("scores_out", [1, S], f32, kind="ExternalOutput")

    enc_tiled = enc[:, :].rearrange("(n p) m -> n p m", p=P)

    with TileContext(nc) as tc:
        with (
            tc.tile_pool(name="io", bufs=1) as io_pool,
            tc.tile_pool(name="consts", bufs=1) as const_pool,
            tc.tile_pool(name="psum", bufs=1, space="PSUM") as psum_pool,
        ):
            # Input DMAs first in program order so the sync HWDGE queue
            # starts streaming immediately. The final tile is loaded as
            # separate column-quarter tiles (Tile tracks dependencies per
            # tile, not per region): each tail quantum then unblocks at its
            # own quarter-DMA's stream end instead of waiting for the whole
            # 512 KB tile to land.
            widths = list(tail_widths) if tail_widths else [H // tail_split] * tail_split
            assert sum(widths) == H
            n_chain = N_TILES - tail_tiles
            # Early chain tiles stream as multi-tile batch DMAs: each DMA
            # carries a fixed ~123ns queue gap, so fewer DMAs shorten the
            # stream. Early tiles don't need per-tile arrival granularity —
            # the add chain has slack — only the last few tiles do.
            plan = list(batch_plan) if batch_plan else [1] * n_chain
            assert sum(plan) == n_chain
            tiles = []  # entries: (lo_ap, hi_ap) per chain tile
            deferred_fulls = []
            pos = 0
            for bi, bsz in enumerate(plan):
                if bsz == 1:
                    t = io_pool.tile([P, H], f32, tag=f"in{pos}")
                    odd = (pos % 2 == 0) if ring_flip else (pos % 2 == 1)
                    eng = nc.scalar if (alt_queues and odd) else nc.sync
                    if pos >= n_chain - early_slices:
                        # Defer the last full(s): their DMAs are emitted
                        # after the tail slices so the slices stream first.
                        deferred_fulls.append((eng, t, pos))
                    else:
                        eng.dma_start(out=t[:, :], in_=enc_tiled[pos])
                    tiles.append((t[:, :split_cols], t[:, split_cols:]))
                else:
                    big = io_pool.tile([P, bsz * H], f32, tag=f"bt{bi}")
                    nc.sync.dma_start(
                        out=big[:, :].rearrange("p (n m) -> p n m", m=H),
                        in_=enc[pos * P : (pos + bsz) * P, :].rearrange(
                            "(n p) m -> p n m", p=P
                        ),
                    )
                    for k in range(bsz):
                        tiles.append(
                            (
                                big[:, k * H : k * H + split_cols],
                                big[:, k * H + split_cols : (k + 1) * H],
                            )
                        )
                pos += bsz
            # The tail tiles stream as per-quantum column slices so each
            # quantum's folds unblock as its own slice lands.
            tail_parts = []  # [tile][quantum]
            for ti in range(n_chain, N_TILES):
                parts = []
                j = 0
                for qi, wd in enumerate(widths):
                    lq = io_pool.tile([P, wd], f32, tag=f"t{ti}q{qi}")
                    if slice_ring_alt == 2:
                        sl_eng = nc.scalar if qi % 2 == 0 else nc.sync
                    else:
                        sl_eng = (
                            nc.scalar if (slice_ring_alt and qi % 2) else nc.sync
                        )
                    sl_eng.dma_start(
                        out=lq[:, :], in_=enc_tiled[ti][:, j : j + wd]
                    )
                    parts.append(lq)
                    j += wd
                tail_parts.append(parts)
            for eng, t, pos_ in deferred_fulls:
                eng.dma_start(out=t[:, :], in_=enc_tiled[pos_])

            # attention scores: exactly 1/S everywhere. Memset on DVE (fast
            # start), DMA out on the scalar-engine HWDGE ring so it never
            # blocks the input stream.
            scores_tile = const_pool.tile([1, S], f32)
            scores_memset_eng.memset(scores_tile[:, :], INV_S)
            nc.scalar.dma_start(out=scores_out[:, :], in_=scores_tile[:, :])

            # ones * (1/S) reduction vector (lhsT for the partition matmul).
            w = const_pool.tile([P, 1], f32)
            scores_memset_eng.memset(w[:, :], INV_S)

            # Serial accumulation chain on the vector engine; each add only
            # needs tile i, so the chain advances as DMAs land. The last
            # tile is folded in per H-quarter in the tail below so the PE
            # can start reducing early quarters while later ones finish.
            # The accumulation chain is column-split across two engines:
            # DVE handles cols [0:split_cols], GPSIMD (otherwise idle)
            # handles [split_cols:H] as an independent serial chain. This
            # halves the per-engine work that remains after the last tiles
            # arrive — DVE alone was the end-of-kernel bottleneck.
            sc = split_cols
            # Separate accumulator tiles per engine: Tile tracks deps at
            # tile granularity, so a shared acc would make every tail fold
            # wait on BOTH chains. Split accumulators let the hi folds start
            # as soon as the (faster) gpsimd chain finishes.
            acc_lo = io_pool.tile([P, sc], f32)
            acc_hi = io_pool.tile([P, H - sc], f32)
            nc.vector.tensor_add(
                out=acc_lo[:, :], in0=tiles[0][0], in1=tiles[1][0]
            )
            nc.gpsimd.tensor_add(
                out=acc_hi[:, :], in0=tiles[0][1], in1=tiles[1][1]
            )
            for i in range(2, n_chain):
                nc.vector.tensor_add(
                    out=acc_lo[:, :], in0=acc_lo[:, :], in1=tiles[i][0]
                )
                nc.gpsimd.tensor_add(
                    out=acc_hi[:, :], in0=acc_hi[:, :], in1=tiles[i][1]
                )

            def acc_slice(lo, hi):
                """AP for acc columns [lo:hi) — must lie in one half."""
                if hi <= sc:
                    return acc_lo[:, lo:hi]
                assert lo >= sc
                return acc_hi[:, lo - sc : hi - sc]

            # PE warm-up: keep the tensor engine busy just before the real
            # reduction matmuls so they run at full clock (HAM ramp). Keyed
            # off a late input tile so the scheduler can't run them early.
            warm_psum = psum_pool.tile([1, warm_cols], f32, tag="warm")
            warm_src = tiles[warm_src_i]
            for _ in range(warm_n):
                nc.tensor.matmul(
                    warm_psum[:, :],
                    lhsT=w[:, :],
                    rhs=warm_src[:, 0:warm_cols],
                    start=True,
                    stop=True,
                )

            # Tail, split over H so the final quantum after the last DMA
            # lands is small: add -> PE partition-reduce -> PSUM copy.
            # Separate PSUM tiles per quantum keep each matmul/copy pair in
            # its own bank, so Tile's bank-overlap tracking never serializes
            # a copy against the next matmul.
            if not col_mm:
                psums = []
                for j, wd in enumerate(widths):
                    pq = psum_pool.tile([1, wd], f32, tag=f"ps{j}")
                    psums.append(pq)
                ctx_sbuf = const_pool.tile([1, H], f32)
            if col_mm:
                n_chunks = H // P  # 8 chunks of 128 columns
                if out_split > 1 and col_mm == 1:
                    n_early = n_chunks - widths[-1] // P
                    psum_col = psum_pool.tile([P, n_early], f32, tag="pscol")
                    psum_col_b = psum_pool.tile(
                        [P, n_chunks - n_early], f32, tag="pscolb"
                    )

                    def psum_slot(c):
                        if c < n_early:
                            return psum_col[:, c : c + 1]
                        return psum_col_b[:, c - n_early : c - n_early + 1]
                else:
                    psum_col = psum_pool.tile([P, n_chunks], f32, tag="pscol")

                    def psum_slot(c):
                        return psum_col[:, c : c + 1]
                if col_mm == 3:
                    # Padded column tile + identity token indices for the
                    # SWDGE scatter-add output path (descriptors generated
                    # early; only a cheap trigger sits after the copy).
                    ctx_col = const_pool.tile([P, 64], f32)
                    nc.vector.memset(ctx_col[:, :], 0.0)
                    idxs = const_pool.tile([128, 8], mybir.dt.int16)
                    nc.gpsimd.iota(
                        idxs[:, :], pattern=[[16, 8]], channel_multiplier=1
                    )
                    scatter_sem = nc.alloc_semaphore("ctx_scatter_dma")
                    nc.gpsimd.dma_scatter_add(
                        context_out[:, :],
                        ctx_col[:, :].rearrange("p (r e) -> p r e", r=1),
                        idxs[:, :],
                        P,
                        P,
                        64,
                        prepare_only=True,
                        sem=scatter_sem,
                    )
                else:
                    ctx_col = const_pool.tile([P, n_chunks], f32)
                offsets = list(np.cumsum([0] + widths[:-1]))
                qorder = tail_order if tail_order is not None else range(len(widths))
                for qi in qorder:
                    j, wd = int(offsets[qi]), widths[qi]
                    if col_mm in (1, 3):
                        # Fold, then per-chunk reduce: out[:, c] = acc_chunk.T @ w
                        fold_eng = (
                            nc.gpsimd
                            if (gpsimd_folds is not None and qi in gpsimd_folds)
                            else nc.vector
                        )
                        fold_eng.tensor_add(
                            out=acc_slice(j, j + wd),
                            in0=acc_slice(j, j + wd),
                            in1=tail_parts[0][qi][:, :],
                        )
                        for c in range(j // P, (j + wd) // P):
                            nc.tensor.matmul(
                                psum_slot(c),
                                lhsT=acc_slice(c * P, (c + 1) * P),
                                rhs=w[:, :],
                                start=True,
                                stop=True,
                            )
                        if per_quantum_copy and col_mm == 1 and out_split == 1:
                            c0, c1 = j // P, (j + wd) // P
                            nc.scalar.copy(
                                out=ctx_col[:, c0:c1], in_=psum_col[:, c0:c1]
                            )
                    else:
                        # No DVE fold: accumulate acc-chunk and last-tile
                        # chunk into the same PSUM column.
                        for c in range(j // P, (j + wd) // P):
                            nc.tensor.matmul(
                                psum_col[:, c : c + 1],
                                lhsT=acc_slice(c * P, (c + 1) * P),
                                rhs=w[:, :],
                                start=True,
                                stop=False,
                            )
                            lo = c * P - j
                            for pi, parts in enumerate(tail_parts):
                                nc.tensor.matmul(
                                    psum_col[:, c : c + 1],
                                    lhsT=parts[qi][:, lo : lo + P],
                                    rhs=w[:, :],
                                    start=False,
                                    stop=(pi == len(tail_parts) - 1),
                                )
                if col_mm == 3:
                    nc.scalar.copy(
                        out=ctx_col[:, :n_chunks], in_=psum_col[:, :]
                    )
                    nc.gpsimd.trigger_dma(count=None)
                elif out_split > 1:
                    # Ship the early chunks while the final quantum's fold
                    # is still pending; separate SBUF tiles keep the first
                    # DMA's dependency off the last copy.
                    ctx_col_b = const_pool.tile([P, n_chunks - n_early], f32)
                    nc.scalar.copy(
                        out=ctx_col[:, :n_early], in_=psum_col[:, :]
                    )
                    nc.sync.dma_start(
                        out=context_out[:, :n_early], in_=ctx_col[:, :n_early]
                    )
                    nc.scalar.copy(out=ctx_col_b[:, :], in_=psum_col_b[:, :])
                    nc.scalar.dma_start(
                        out=context_out[:, n_early:n_chunks], in_=ctx_col_b[:, :]
                    )
                elif per_quantum_copy:
                    nc.sync.dma_start(out=context_out[:, :], in_=ctx_col[:, :])
                else:
                    nc.scalar.copy(out=ctx_col[:, :], in_=psum_col[:, :])
                    nc.sync.dma_start(out=context_out[:, :], in_=ctx_col[:, :])

            offsets = list(np.cumsum([0] + widths[:-1]))
            order = tail_order if tail_order is not None else range(len(widths))
            for qi in order if not col_mm else []:
                j, wd = int(offsets[qi]), widths[qi]
                if gpsimd_folds is not None:
                    on_gpsimd = qi in gpsimd_folds
                else:
                    # Quanta fully inside the GPSIMD column range fold there.
                    on_gpsimd = j >= split_cols
                fold_eng = nc.gpsimd if on_gpsimd else nc.vector
                for parts in tail_parts:
                    fold_eng.tensor_add(
                        out=acc_slice(j, j + wd),
                        in0=acc_slice(j, j + wd),
                        in1=parts[qi][:, :],
                    )
                # Partition-dim reduction: context = (ones/S).T @ acc.
                nc.tensor.matmul(
                    psums[qi][:, :],
                    lhsT=w[:, :],
                    rhs=acc_slice(j, j + wd),
                    start=True,
                    stop=True,
                )
                if last_copy_dve and qi == len(widths) - 1:
                    nc.vector.tensor_copy(
                        out=ctx_sbuf[:, j : j + wd], in_=psums[qi][:, :]
                    )
                else:
                    nc.scalar.copy(out=ctx_sbuf[:, j : j + wd], in_=psums[qi][:, :])
            if not col_mm:
                out_eng = nc.scalar if out_on_act else nc.sync
                if out_split > 1:
                    cut = H - widths[-1]
                    nc.sync.dma_start(
                        out=context_out[:, :cut], in_=ctx_sbuf[:, :cut]
                    )
                    nc.scalar.dma_start(
                        out=context_out[:, cut:], in_=ctx_sbuf[:, cut:]
                    )
                else:
                    out_eng.dma_start(out=context_out[:, :], in_=ctx_sbuf[:, :])

    nc.finalize()
    return nc


def _get_nc():
    global _NC_CACHE
    if _NC_CACHE is None:
        _NC_CACHE = _build_nc()
    return _NC_CACHE


_EXEC_CACHE = None


def _get_exec():
    """Cached jitted SPMD executable (one trace/compile per process).

    Mirrors concourse.bass2jax.run_bass_via_pjrt's multi-core path, but
    reuses the jitted callable across kernel() calls — run_bass_via_pjrt
    builds a fresh closure per call, retracing and recompiling every time.
    """
    global _EXEC_CACHE
    if _EXEC_CACHE is not None:
        return _EXEC_CACHE

    import jax
    from jax.sharding import Mesh, PartitionSpec
    from jax.experimental.shard_map import shard_map

    import concourse.mybir as mybir_
    from concourse import bass2jax

    nc = _get_nc()
    bass2jax.install_neuronx_cc_hook()

    partition_name = nc.partition_id_tensor.name if nc.partition_id_tensor else None
    in_names, out_names, out_avals, zero_out_shapes = [], [], [], []
    for alloc in nc.m.functions[0].allocations:
        if not isinstance(alloc, mybir_.MemoryLocationSet):
            continue
        name = alloc.memorylocations[0].name
        if alloc.kind == "ExternalInput":
            if name != partition_name:
                in_names.append(name)
        elif alloc.kind == "ExternalOutput":
            shape = tuple(alloc.tensor_shape)
            dtype = mybir_.dt.np(alloc.dtype)
            out_names.append(name)
            out_avals.append(jax.core.ShapedArray(shape, dtype))
            zero_out_shapes.append((shape, dtype))
    n_params = len(in_names)
    all_names = list(in_names) + list(out_names)
    if partition_name is not None:
        all_names.append(partition_name)

    def _body(*args):
        operands = list(args)
        if partition_name is not None:
            operands.append(bass2jax.partition_id_tensor())
        outs = bass2jax._bass_exec_p.bind(
            *operands,
            out_avals=tuple(out_avals),
            in_names=tuple(all_names),
            out_names=tuple(out_names),
            lowering_input_output_aliases=(),
            sim_require_finite=True,
            sim_require_nnan=True,
            nc=nc,
        )
        return tuple(outs)

    devices = jax.devices()[:B]
    mesh = Mesh(np.asarray(devices), ("core",))
    n_outs = len(out_names)
    sharded = jax.jit(
        shard_map(
            _body,
            mesh=mesh,
            in_specs=(PartitionSpec("core"),) * (n_params + n_outs),
            out_specs=(PartitionSpec("core"),) * n_outs,
            check_rep=False,
        ),
        donate_argnums=tuple(range(n_params, n_params + n_outs)),
        keep_unused=True,
    )
    _EXEC_CACHE = (sharded, in_names, out_names, zero_out_shapes)
    return _EXEC_CACHE


def kernel(**inputs) -> tuple[np.ndarray, np.ndarray]:
    enc = np.ascontiguousarray(np.asarray(inputs["encoder_outputs"], dtype=np.float32))
    assert enc.shape == (B, S, H)

    sharded, in_names, out_names, zero_out_shapes = _get_exec()
    assert in_names == ["enc"]
    concat_in = [enc.reshape(B * S, H)]
    concat_zeros = [
        np.zeros((B * shape[0], *shape[1:]), dtype) for shape, dtype in zero_out_shapes
    ]
    out_arrs = sharded(*concat_in, *concat_zeros)
    outs = {}
    for i, name in enumerate(out_names):
        shape, _ = zero_out_shapes[i]
        arr = np.asarray(out_arrs[i]).reshape(B, *shape)
        if name == "context_out" and shape != (1, H):
            # Column layout from the swapped-operand PE reduce (possibly
            # padded for the scatter-add path): arr[b, p, c] = context[b,
            # c*128 + p] for p < 128, c < 8.
            arr = np.ascontiguousarray(arr[:, :P, : H // P].transpose(0, 2, 1))
        outs[name] = arr.reshape(B, -1)
    return outs["context_out"], outs["scores_out"]
